# revision 19
# baseline (speedup 1.0000x reference)
import sys

sys.path.insert(0, "/opt/trn_rl_repo")
import numpy as np
import concourse.bass as bass
import concourse.tile as tile
from concourse import bacc, mybir
from concourse.alu_op_type import AluOpType
from concourse.bass_utils import run_bass_kernel_spmd

# Problem constants (nn_EquivGNNEncoder: 2048 graphs x 32 atoms, 3 layers)
B, NA = 2048, 32
N = B * NA                  # 65536 nodes
S_MUL, V_MUL = 32, 16
NCORES = 8
GPC = B // NCORES           # 256 graphs per core
NPC = GPC * NA              # 8192 nodes per core
GPB = 4                     # graphs per block (4*32 = 128 partitions)
NBLK = GPC // GPB           # 64 blocks per core
LAT = 128                   # latent out dim
HID = 256
NPAIR = NBLK // 2           # 32 block-pairs per core

INV_SQRT3 = 1.0 / np.sqrt(3.0)
C_SCALAR = np.float32(1.0 / np.sqrt(48.0))
C_VECTOR = np.float32(np.sqrt(3.0 / 48.0))

F32 = mybir.dt.float32
F32R = mybir.dt.float32r
BF16 = mybir.dt.bfloat16

_CACHE = {}

# node feature column layout: [s(0:32) | vx(32:48) | vy(48:64) | vz(64:80)]
# geometry via PE, two matmuls per block into PSUM [128, 512]:
#   cols 0:384  diff[i, c*128+j] = pos[j,c] - pos[i,c]   (bf16 hi+lo split:
#     lhsT GL [8,128] = [1, 1, pxh, pxl, pyh, pyl, pzh, pzl],
#     rhs GR [8,384]: r0 = pjc_hi, r1 = pjc_lo, r2+2c/r3+2c = -delta_c)
#   cols 384:512 d2[i, j] = |pi|^2 + |pj|^2 - 2 pi.pj    (exact fp32:
#     lhsT GL2 [5,128] = [1, px, py, pz, |p|^2],
#     rhs GR2 [5,128]: [|pj|^2, -2pjx, -2pjy, -2pjz, 1])
# gm pair tile [128, 1024], block h at h*512: [mask(128) | shx | shy | shz]
# ps_agg pair [128, 1024]; repack per pair-layer as TWO merged 4D copies:
#   AC [128, 512] = [ssaA | ssaB | sscA | sscB]   (mask cols, shz cols)
#   SB [64, 512]  = [shxA | shxB | shyA | shyB]   (rows 0:64 of shx/shy)
# transform = 4 matmuls per block: AC-ssa @ wta, AC-ssc @ wtc,
#   SB-shx @ wtbx, SB-shy @ wtby (64-row contraction each)


def _build_program():
    nc = bacc.Bacc("TRN2", target_bir_lowering=False, debug=False)

    s0_ap = nc.dram_tensor("s0", [NPC, S_MUL], BF16, kind="ExternalInput").ap()
    gl_ap = nc.dram_tensor("gl", [NBLK, 8, 128], BF16, kind="ExternalInput").ap()
    gr_ap = nc.dram_tensor("gr", [NBLK, 8, 384], BF16, kind="ExternalInput").ap()
    gl2_ap = nc.dram_tensor("gl2", [NBLK, 5, 128], F32, kind="ExternalInput").ap()
    gr2_ap = nc.dram_tensor("gr2", [NBLK, 5, 128], F32, kind="ExternalInput").ap()
    bd_ap = nc.dram_tensor("bd", [128, 128], F32, kind="ExternalInput").ap()
    wta_ap = nc.dram_tensor("wta", [3, 128, 80], F32, kind="ExternalInput").ap()
    wtbx_ap = nc.dram_tensor("wtbx", [3, 64, 80], F32, kind="ExternalInput").ap()
    wtby_ap = nc.dram_tensor("wtby", [3, 64, 80], F32, kind="ExternalInput").ap()
    wtc_ap = nc.dram_tensor("wtc", [3, 128, 80], F32, kind="ExternalInput").ap()
    poolm_ap = nc.dram_tensor("poolm", [128, GPB], F32, kind="ExternalInput").ap()
    wr1_ap = nc.dram_tensor("wr1", [80, HID], F32, kind="ExternalInput").ap()
    br1_ap = nc.dram_tensor("br1", [HID, 1], F32, kind="ExternalInput").ap()
    wr2_ap = nc.dram_tensor("wr2", [HID, LAT], F32, kind="ExternalInput").ap()
    br2_ap = nc.dram_tensor("br2", [LAT, 1], F32, kind="ExternalInput").ap()
    out_ap = nc.dram_tensor("outfm", [LAT, GPC], F32, kind="ExternalOutput").ap()

    with tile.TileContext(nc) as tc:
        with tc.tile_pool(name="const", bufs=1) as const, \
             tc.tile_pool(name="stage", bufs=3) as stage, \
             tc.tile_pool(name="gmp", bufs=5) as gmp, \
             tc.tile_pool(name="featb", bufs=10, space="SBUF") as featbp, \
             tc.tile_pool(name="ssp", bufs=8) as ssp, \
             tc.tile_pool(name="work", bufs=8) as work, \
             tc.tile_pool(name="psgeo", bufs=1, space="PSUM") as psp_geo, \
             tc.tile_pool(name="psagg", bufs=2, space="PSUM") as psp_agg, \
             tc.tile_pool(name="psh", bufs=1, space="PSUM") as psp_h, \
             tc.tile_pool(name="pspool", bufs=1, space="PSUM") as psp_pool:

            # --- constants ---
            bd = const.tile([128, 128], F32)
            nc.sync.dma_start(bd[:], bd_ap[:])
            wts = []
            for nm, ap_, rows in (("a", wta_ap, 128), ("bx", wtbx_ap, 64),
                                  ("by", wtby_ap, 64), ("c", wtc_ap, 128)):
                wf = const.tile([rows, 3, 80], F32, tag="wf" + nm)
                nc.scalar.dma_start(
                    wf[:],
                    bass.AP(tensor=ap_.tensor, offset=ap_.offset,
                            ap=[[80, rows], [rows * 80, 3], [1, 80]]),
                )
                wb = const.tile([rows, 3, 80], BF16, tag="wb" + nm)
                nc.vector.tensor_copy(wb[:], wf[:])
                wts.append(wb)
            wta, wtbx, wtby, wtc = wts
            poolm_f = const.tile([128, GPB], F32)
            nc.sync.dma_start(poolm_f[:], poolm_ap[:])
            poolm = const.tile([128, GPB], BF16)
            nc.vector.tensor_copy(poolm[:], poolm_f[:])
            wr1_f = const.tile([80, HID], F32)
            nc.scalar.dma_start(wr1_f[:], wr1_ap[:])
            wr1 = const.tile([80, HID], F32R)
            nc.vector.tensor_copy(wr1[:], wr1_f[:])
            wr2a_f = const.tile([128, LAT], F32)
            nc.scalar.dma_start(wr2a_f[:], wr2_ap[0:128, :])
            wr2a = const.tile([128, LAT], F32R)
            nc.vector.tensor_copy(wr2a[:], wr2a_f[:])
            wr2b_f = const.tile([128, LAT], F32)
            nc.scalar.dma_start(wr2b_f[:], wr2_ap[128:256, :])
            wr2b = const.tile([128, LAT], F32R)
            nc.vector.tensor_copy(wr2b[:], wr2b_f[:])
            br1a = const.tile([128, 1], F32)
            nc.sync.dma_start(br1a[:], br1_ap[0:128, :])
            br1b = const.tile([128, 1], F32)
            nc.sync.dma_start(br1b[:], br1_ap[128:256, :])
            br2 = const.tile([LAT, 1], F32)
            nc.sync.dma_start(br2[:], br2_ap[:])
            epsb = const.tile([128, 1], F32)
            nc.vector.memset(epsb[:], 1e-4 / 3.0)

            # pooled per-graph features, feature-major [80, 256]
            xfm = const.tile([80, GPC], F32R)

            # prime the featb ring: zero FWL-pad cols {80:128, 208:256} once
            for _ in range(10):
                t = featbp.tile([128, 256], BF16, tag="fb")
                nc.vector.memset(t[:, 80:128], 0.0)
                nc.vector.memset(t[:, 208:256], 0.0)

            GRPP = 4   # pairs per group

            def emit_group_stage(g):
                # one DMA each for the group's geometry lhsT/rhs tensors
                NB8 = GRPP * 2
                glg = stage.tile([8, NB8 * 128], BF16, tag="glg")
                nc.scalar.dma_start(
                    glg[:],
                    bass.AP(tensor=gl_ap.tensor,
                            offset=gl_ap.offset + g * NB8 * 8 * 128,
                            ap=[[128, 8], [8 * 128, NB8], [1, 128]]),
                )
                grg = stage.tile([8, NB8 * 384], BF16, tag="grg")
                nc.scalar.dma_start(
                    grg[:],
                    bass.AP(tensor=gr_ap.tensor,
                            offset=gr_ap.offset + g * NB8 * 8 * 384,
                            ap=[[384, 8], [8 * 384, NB8], [1, 384]]),
                )
                gl2g = stage.tile([5, NB8 * 128], F32, tag="gl2g")
                nc.sync.dma_start(
                    gl2g[:],
                    bass.AP(tensor=gl2_ap.tensor,
                            offset=gl2_ap.offset + g * NB8 * 5 * 128,
                            ap=[[128, 5], [5 * 128, NB8], [1, 128]]),
                )
                gr2g = stage.tile([5, NB8 * 128], F32, tag="gr2g")
                nc.sync.dma_start(
                    gr2g[:],
                    bass.AP(tensor=gr2_ap.tensor,
                            offset=gr2_ap.offset + g * NB8 * 5 * 128,
                            ap=[[128, 5], [5 * 128, NB8], [1, 128]]),
                )
                return glg, grg, gl2g, gr2g

            def emit_geo(p, i, gs):
                # p: global pair idx, i: pair-in-group idx
                glg, grg, gl2g, gr2g = gs
                pg = psp_geo.tile([128, 1024], F32, tag="pg")
                for h in range(2):
                    b = 2 * i + h
                    nc.tensor.matmul(
                        pg[:, h * 512:h * 512 + 384],
                        glg[:, b * 128:(b + 1) * 128],
                        grg[:, b * 384:(b + 1) * 384],
                        start=True, stop=True)
                    nc.tensor.matmul(
                        pg[:, h * 512 + 384:(h + 1) * 512],
                        gl2g[:, b * 128:(b + 1) * 128],
                        gr2g[:, b * 128:(b + 1) * 128],
                        start=True, stop=True)

                gm = gmp.tile([128, 1024], BF16, tag="gm")
                # mask for both blocks: is_le(d2, 25) * bd
                nc.vector.scalar_tensor_tensor(
                    bass.AP(tensor=gm.tensor, offset=gm.offset,
                            ap=[[gm.shape[1], 128], [512, 2], [1, 128]]),
                    bass.AP(tensor=pg.tensor, offset=pg.offset + 384,
                            ap=[[pg.shape[1], 128], [512, 2], [1, 128]]),
                    25.0,
                    bass.AP(tensor=bd.tensor, offset=bd.offset,
                            ap=[[bd.shape[1], 128], [0, 2], [1, 128]]),
                    AluOpType.is_le, AluOpType.mult)
                # s3 = sqrt(d2/3 + eps), rs = 1/s3 = sqrt(3)/|r|
                s3 = work.tile([128, 256], F32, tag="s3")
                nc.scalar.activation(
                    bass.AP(tensor=s3.tensor, offset=s3.offset,
                            ap=[[s3.shape[1], 128], [128, 2], [1, 128]]),
                    bass.AP(tensor=pg.tensor, offset=pg.offset + 384,
                            ap=[[pg.shape[1], 128], [512, 2], [1, 128]]),
                    mybir.ActivationFunctionType.Sqrt,
                    bias=epsb[:], scale=float(1.0 / 3.0))
                rs = work.tile([128, 256], F32, tag="rs")
                nc.vector.reciprocal_approx_fast(rs[:], s3[:])
                # ga = rs * mask
                ga = work.tile([128, 256], F32, tag="ga")
                nc.gpsimd.tensor_mul(
                    ga[:], rs[:],
                    bass.AP(tensor=gm.tensor, offset=gm.offset,
                            ap=[[gm.shape[1], 128], [512, 2], [1, 128]]))
                # diff PSUM -> SBUF bf16 (scalar), then gm_sh = diff * ga on
                # gpsimd (bcast ga over c), one 4D op for the pair
                diffs = work.tile([128, 768], BF16, tag="diffs")
                nc.scalar.copy(
                    bass.AP(tensor=diffs.tensor, offset=diffs.offset,
                            ap=[[diffs.shape[1], 128], [384, 2], [1, 384]]),
                    bass.AP(tensor=pg.tensor, offset=pg.offset,
                            ap=[[pg.shape[1], 128], [512, 2], [1, 384]]))
                nc.gpsimd.tensor_mul(
                    bass.AP(tensor=gm.tensor, offset=gm.offset + 128,
                            ap=[[gm.shape[1], 128], [512, 2], [128, 3], [1, 128]]),
                    bass.AP(tensor=diffs.tensor, offset=diffs.offset,
                            ap=[[diffs.shape[1], 128], [384, 2], [128, 3], [1, 128]]),
                    bass.AP(tensor=ga.tensor, offset=ga.offset,
                            ap=[[ga.shape[1], 128], [128, 2], [0, 3], [1, 128]]))

                # node features bf16 pair tile; FWL pad cols primed zero
                featb = featbp.tile([128, 256], BF16, tag="fb")
                nc.gpsimd.memset(featb[:, 32:80], 0.0)
                nc.gpsimd.memset(featb[:, 160:208], 0.0)
                nc.sync.dma_start(
                    bass.AP(tensor=featb.tensor, offset=featb.offset,
                            ap=[[featb.shape[1], 128], [128, 2], [1, 32]]),
                    bass.AP(tensor=s0_ap.tensor,
                            offset=s0_ap.offset + p * 256 * S_MUL,
                            ap=[[S_MUL, 128], [128 * S_MUL, 2], [1, S_MUL]]),
                )
                return gm, featb

            def emit_agg(gm, featb):
                pa = psp_agg.tile([128, 1024], F32, tag="agg")
                for h in range(2):
                    nc.tensor.matmul(pa[:, h * 512:(h + 1) * 512],
                                     featb[:, h * 128:(h + 1) * 128],
                                     gm[:, h * 512:(h + 1) * 512],
                                     start=True, stop=True)
                return pa

            def emit_copies(pa):
                # two merged 4D repack copies, PSUM -> SBUF bf16
                # AC [128, 512] = [ssaA | ssaB | sscA | sscB]
                # SB [64, 512]  = [shxA | shxB | shyA | shyB]
                ac = ssp.tile([128, 512], BF16, tag="ac")
                sb = ssp.tile([64, 512], BF16, tag="sb")
                nc.vector.tensor_copy(
                    bass.AP(tensor=ac.tensor, offset=ac.offset,
                            ap=[[ac.shape[1], 128], [256, 2], [128, 2], [1, 128]]),
                    bass.AP(tensor=pa.tensor, offset=pa.offset,
                            ap=[[pa.shape[1], 128], [384, 2], [512, 2], [1, 128]]))
                nc.scalar.copy(
                    bass.AP(tensor=sb.tensor, offset=sb.offset,
                            ap=[[sb.shape[1], 64], [256, 2], [128, 2], [1, 128]]),
                    bass.AP(tensor=pa.tensor, offset=pa.offset + 128,
                            ap=[[pa.shape[1], 64], [128, 2], [512, 2], [1, 128]]))
                return ac, sb

            def emit_transform(l, ss):
                ac, sb = ss
                ph = psp_h.tile([128, 160], F32, tag="psh")
                for h in range(2):
                    ps = ph[:, h * 80:(h + 1) * 80]
                    nc.tensor.matmul(ps, ac[:, h * 128:(h + 1) * 128],
                                     wta[:, l, :], start=True, stop=False)
                    nc.tensor.matmul(ps, ac[:, 256 + h * 128:256 + (h + 1) * 128],
                                     wtc[:, l, :], start=False, stop=False)
                    nc.tensor.matmul(ps, sb[:, h * 128:(h + 1) * 128],
                                     wtbx[:, l, :], start=False, stop=False)
                    nc.tensor.matmul(ps, sb[:, 256 + h * 128:256 + (h + 1) * 128],
                                     wtby[:, l, :], start=False, stop=True)
                return ph

            def emit_resid(ph, featb):
                featbn = featbp.tile([128, 256], BF16, tag="fb")
                nc.vector.scalar_tensor_tensor(
                    bass.AP(tensor=featbn.tensor, offset=featbn.offset,
                            ap=[[featbn.shape[1], 128], [128, 2], [1, 80]]),
                    bass.AP(tensor=ph.tensor, offset=ph.offset,
                            ap=[[ph.shape[1], 128], [80, 2], [1, 80]]),
                    0.0,
                    bass.AP(tensor=featb.tensor, offset=featb.offset,
                            ap=[[featb.shape[1], 128], [128, 2], [1, 80]]),
                    AluOpType.max, AluOpType.add)
                return featbn

            def emit_pool(pp, i, featb):
                for h in range(2):
                    nc.tensor.matmul(
                        pp[:, (2 * i + h) * GPB:(2 * i + h + 1) * GPB],
                        featb[:, h * 128:(h + 1) * 128],
                        poolm[:], start=True, stop=True)

            NGRP = NPAIR // GRPP     # 8 groups
            for g in range(NGRP):
                gs = emit_group_stage(g)
                st = [emit_geo(g * GRPP + i, i, gs) for i in range(GRPP)]
                for l in range(3):
                    pas = [emit_agg(st[i][0], st[i][1]) for i in range(GRPP)]
                    sss = [emit_copies(pas[i]) for i in range(GRPP)]
                    for i in range(GRPP):
                        ph = emit_transform(l, sss[i])
                        st[i] = (st[i][0], emit_resid(ph, st[i][1]))
                pp = psp_pool.tile([128, GRPP * 2 * GPB], F32, tag="pool")
                for i in range(GRPP):
                    emit_pool(pp, i, st[i][1])
                nc.vector.tensor_copy(
                    xfm[0:80, g * GRPP * 2 * GPB:(g + 1) * GRPP * 2 * GPB],
                    pp[0:80, :])

            # --- readout MLP: relu(x @ Wr1 + br1) @ Wr2 + br2, feature-major ---
            t1 = psp_agg.tile([128, 1024], F32, tag="agg")
            t2 = psp_agg.tile([128, 1024], F32, tag="agg")
            ps_h1 = t1[:, 0:GPC]
            ps_h2 = t2[:, 0:GPC]
            nc.tensor.matmul(ps_h1, wr1[:, 0:128], xfm[:], start=True, stop=True)
            nc.tensor.matmul(ps_h2, wr1[:, 128:256], xfm[:], start=True, stop=True)
            hid1 = work.tile([128, GPC], F32R, tag="hid1")
            hid2 = work.tile([128, GPC], F32R, tag="hid2")
            nc.vector.tensor_scalar(hid1[:], ps_h1, br1a[:], 0.0,
                                    AluOpType.add, AluOpType.max)
            nc.vector.tensor_scalar(hid2[:], ps_h2, br1b[:], 0.0,
                                    AluOpType.add, AluOpType.max)
            t3 = psp_agg.tile([128, 1024], F32, tag="agg")
            ps_o = t3[0:LAT, 0:GPC]
            nc.tensor.matmul(ps_o, wr2a[:], hid1[:], start=True, stop=False)
            nc.tensor.matmul(ps_o, wr2b[:], hid2[:], start=False, stop=True)
            outt = work.tile([LAT, GPC], F32, tag="outt")
            nc.vector.tensor_scalar(outt[:], ps_o, br2[:], None,
                                    AluOpType.add)
            nc.sync.dma_start(out_ap[:], outt[:])

    nc.compile()
    return nc


def kernel(pos, emb, W_s2n, W1, W2, W3, W4, Ws, Wv, Wr1, br1, Wr2, br2,
           z, batch, edge_index, num_graphs):
    pos = np.asarray(pos, dtype=np.float32)
    z = np.asarray(z)
    emb = np.asarray(emb, dtype=np.float32)
    W_s2n = np.asarray(W_s2n, dtype=np.float32)
    W1 = np.asarray(W1, dtype=np.float32); W2 = np.asarray(W2, dtype=np.float32)
    W3 = np.asarray(W3, dtype=np.float32); W4 = np.asarray(W4, dtype=np.float32)
    Ws = np.asarray(Ws, dtype=np.float32); Wv = np.asarray(Wv, dtype=np.float32)
    Wr1 = np.asarray(Wr1, dtype=np.float32); br1 = np.asarray(br1, dtype=np.float32)
    Wr2 = np.asarray(Wr2, dtype=np.float32); br2 = np.asarray(br2, dtype=np.float32)

    # host prep: embedding lookup folded with input linear
    import ml_dtypes
    EW = (emb @ W_s2n) * np.float32(1.0 / np.sqrt(S_MUL))     # [100, 32]
    s0 = EW[z].astype(ml_dtypes.bfloat16)                     # [N, 32] bf16

    # fused transform weights with norm constants folded in (rows padded to 128)
    cs = C_SCALAR * np.float32(1.0 / np.sqrt(S_MUL))
    csb = C_SCALAR * np.float32(INV_SQRT3 / np.sqrt(S_MUL))
    cv = C_VECTOR * np.float32(INV_SQRT3 / np.sqrt(V_MUL))
    wta = np.zeros((3, 128, 80), np.float32)
    wtbx = np.zeros((3, 64, 80), np.float32)
    wtby = np.zeros((3, 64, 80), np.float32)
    wtc = np.zeros((3, 128, 80), np.float32)
    for l in range(3):
        Wa = cs * (W1[l] @ Ws[l])        # [32,32] s_m -> s
        Wb = csb * (W4[l] @ Ws[l])       # [16,32] v_c*sh_c -> s
        Wc = cv * (W2[l] @ Wv[l])        # [32,16] s*sh_c -> v_c
        Wd = cv * (W3[l] @ Wv[l])        # [16,16] v_c_m -> v_c
        # AC-ssa rows: [s_m(0:32) vx_m(32:48) vy_m(48:64) vz_m(64:80)]
        wta[l, 0:32, 0:32] = Wa
        wta[l, 32:48, 32:48] = Wd
        wta[l, 48:64, 48:64] = Wd
        wta[l, 64:80, 64:80] = Wd
        # SB-shx rows: [s_x(0:32) vxx(32:48) junk(48:64)]
        wtbx[l, 0:32, 32:48] = Wc
        wtbx[l, 32:48, 0:32] = Wb
        # SB-shy rows: [s_y(0:32) junk(32:48) vyy(48:64)]
        wtby[l, 0:32, 48:64] = Wc
        wtby[l, 48:64, 0:32] = Wb
        # AC-ssc rows: [s_z(0:32) junk(32:64) vzz(64:80)]
        wtc[l, 0:32, 64:80] = Wc
        wtc[l, 64:80, 0:32] = Wb

    # readout Wr1 with rows permuted to the [s | vx | vy | vz] feature order
    wr1p = np.zeros((80, HID), np.float32)
    wr1p[0:32] = Wr1[0:32]                        # s
    for u in range(V_MUL):
        wr1p[32 + u] = Wr1[S_MUL + 3 * u + 0]     # vx
        wr1p[48 + u] = Wr1[S_MUL + 3 * u + 1]     # vy
        wr1p[64 + u] = Wr1[S_MUL + 3 * u + 2]     # vz

    bdm = np.zeros((128, 128), np.float32)
    for g in range(GPB):
        bdm[g * NA:(g + 1) * NA, g * NA:(g + 1) * NA] = 1.0
    np.fill_diagonal(bdm, 0.0)                    # no self-loops (d2 > 0)
    poolm = np.zeros((128, GPB), np.float32)
    for g in range(GPB):
        poolm[g * NA:(g + 1) * NA, g] = 1.0

    if "nc" not in _CACHE:
        _CACHE["nc"] = _build_program()
    nc = _CACHE["nc"]

    in_maps = []
    for c in range(NCORES):
        psl = pos[c * NPC:(c + 1) * NPC]                       # [8192, 3]
        pb = psl.reshape(NBLK, 128, 3)                         # [64, 128, 3]
        nrm2 = (pb * pb).sum(-1)                               # [64, 128]
        pbt = pb.transpose(0, 2, 1)                            # [64, 3, 128]
        ph = pbt.astype(ml_dtypes.bfloat16)                    # hi bf16
        pl = (pbt - ph.astype(np.float32)).astype(ml_dtypes.bfloat16)
        gl = np.zeros((NBLK, 8, 128), ml_dtypes.bfloat16)
        gl[:, 0, :] = 1.0
        gl[:, 1, :] = 1.0
        for cc in range(3):
            gl[:, 2 + 2 * cc, :] = ph[:, cc]
            gl[:, 3 + 2 * cc, :] = pl[:, cc]
        gr = np.zeros((NBLK, 8, 384), ml_dtypes.bfloat16)
        for cc in range(3):
            gr[:, 0, cc * 128:(cc + 1) * 128] = ph[:, cc]
            gr[:, 1, cc * 128:(cc + 1) * 128] = pl[:, cc]
            gr[:, 2 + 2 * cc, cc * 128:(cc + 1) * 128] = -1.0
            gr[:, 3 + 2 * cc, cc * 128:(cc + 1) * 128] = -1.0
        gl2 = np.empty((NBLK, 5, 128), np.float32)
        gl2[:, 0, :] = 1.0
        gl2[:, 1:4, :] = pbt
        gl2[:, 4, :] = nrm2
        gr2 = np.empty((NBLK, 5, 128), np.float32)
        gr2[:, 0, :] = nrm2
        gr2[:, 1:4, :] = -2.0 * pbt
        gr2[:, 4, :] = 1.0
        in_maps.append(dict(
            s0=np.ascontiguousarray(s0[c * NPC:(c + 1) * NPC]),
            gl=gl, gr=gr, gl2=gl2, gr2=gr2,
            bd=bdm, wta=wta, wtbx=wtbx, wtby=wtby, wtc=wtc, poolm=poolm,
            wr1=wr1p, br1=br1.reshape(HID, 1),
            wr2=Wr2, br2=br2.reshape(LAT, 1),
        ))

    res = run_bass_kernel_spmd(nc, in_maps, core_ids=list(range(NCORES)))
    out = np.empty((B, LAT), np.float32)
    for c in range(NCORES):
        out[c * GPC:(c + 1) * GPC] = res.results[c]["outfm"].T
    return out


# revision 23
# speedup vs baseline: 1.1931x; 1.1931x over previous
import sys

sys.path.insert(0, "/opt/trn_rl_repo")
import numpy as np
import concourse.bass as bass
import concourse.tile as tile
from concourse import bacc, mybir
from concourse.alu_op_type import AluOpType
from concourse.bass_utils import run_bass_kernel_spmd

# Problem constants (nn_EquivGNNEncoder: 2048 graphs x 32 atoms, 3 layers)
B, NA = 2048, 32
N = B * NA                  # 65536 nodes
S_MUL, V_MUL = 32, 16
NCORES = 8
GPC = B // NCORES           # 256 graphs per core
NPC = GPC * NA              # 8192 nodes per core
GPB = 4                     # graphs per block (4*32 = 128 partitions)
NBLK = GPC // GPB           # 64 blocks per core
LAT = 128                   # latent out dim
HID = 256
NPAIR = NBLK // 2           # 32 block-pairs per core

INV_SQRT3 = 1.0 / np.sqrt(3.0)
C_SCALAR = np.float32(1.0 / np.sqrt(48.0))
C_VECTOR = np.float32(np.sqrt(3.0 / 48.0))

F32 = mybir.dt.float32
F32R = mybir.dt.float32r
BF16 = mybir.dt.bfloat16

_CACHE = {}

# node feature column layout: [s(0:32) | vx(32:48) | vy(48:64) | vz(64:80)]
# geometry: ONE bf16 matmul per block -> PSUM [128, 512]:
#   cols 0:384  diff[i, c*128+j] = pos[j,c]-pos[i,c]  (3-way bf16 split, exact
#     to ~2^-24); cols 384:512 d2[i,j] (split products, err ~1e-4)
# gm pair tile [128, 1024], block h at h*512: [mask(128) | shx | shy | shz]
# ps_agg pair [80, 1024] (lhsT = featb 80 cols); repack per pair-layer:
#   ssa[0:80]   <- PA[0:80, mask]
#   ssb[0:64]   <- PA[0:64, shy]   (s_y, vx junk, vyy)
#   ssb[64:112] <- PA[0:48, shx]   (s_x, vxx)
#   ssc[0:80]   <- PA[0:80, shz]   (s_z, junk, vzz)
# transform = 3 matmuls per block (wta 80, wtb 112, wtc 80 rows)


def _build_program():
    nc = bacc.Bacc("TRN2", target_bir_lowering=False, debug=False)

    s0_ap = nc.dram_tensor("s0", [NPC, S_MUL], BF16, kind="ExternalInput").ap()
    gl_ap = nc.dram_tensor("gl", [NBLK, 24, 128], BF16, kind="ExternalInput").ap()
    gr_ap = nc.dram_tensor("gr", [NBLK, 24, 512], BF16, kind="ExternalInput").ap()
    bd_ap = nc.dram_tensor("bd", [128, 128], F32, kind="ExternalInput").ap()
    wta_ap = nc.dram_tensor("wta", [3, 80, 80], F32, kind="ExternalInput").ap()
    wtb_ap = nc.dram_tensor("wtb", [3, 112, 80], F32, kind="ExternalInput").ap()
    wtc_ap = nc.dram_tensor("wtc", [3, 80, 80], F32, kind="ExternalInput").ap()
    poolm_ap = nc.dram_tensor("poolm", [128, GPB], F32, kind="ExternalInput").ap()
    wr1_ap = nc.dram_tensor("wr1", [80, HID], F32, kind="ExternalInput").ap()
    br1_ap = nc.dram_tensor("br1", [HID, 1], F32, kind="ExternalInput").ap()
    wr2_ap = nc.dram_tensor("wr2", [HID, LAT], F32, kind="ExternalInput").ap()
    br2_ap = nc.dram_tensor("br2", [LAT, 1], F32, kind="ExternalInput").ap()
    out_ap = nc.dram_tensor("outfm", [LAT, GPC], F32, kind="ExternalOutput").ap()

    with tile.TileContext(nc) as tc:
        with tc.tile_pool(name="const", bufs=1) as const, \
             tc.tile_pool(name="stage", bufs=3) as stage, \
             tc.tile_pool(name="gmp", bufs=5) as gmp, \
             tc.tile_pool(name="featb", bufs=10, space="SBUF") as featbp, \
             tc.tile_pool(name="ssp", bufs=8) as ssp, \
             tc.tile_pool(name="work", bufs=6) as work, \
             tc.tile_pool(name="psgeo", bufs=1, space="PSUM") as psp_geo, \
             tc.tile_pool(name="psagg", bufs=3, space="PSUM") as psp_agg, \
             tc.tile_pool(name="psh", bufs=1, space="PSUM") as psp_h:

            # --- constants ---
            bd = const.tile([128, 128], F32)
            nc.sync.dma_start(bd[:], bd_ap[:])
            wts = []
            for nm, ap_, rows in (("a", wta_ap, 80), ("b", wtb_ap, 112),
                                  ("c", wtc_ap, 80)):
                wf = const.tile([rows, 3, 80], F32, tag="wf" + nm)
                nc.scalar.dma_start(
                    wf[:],
                    bass.AP(tensor=ap_.tensor, offset=ap_.offset,
                            ap=[[80, rows], [rows * 80, 3], [1, 80]]),
                )
                wb = const.tile([rows, 3, 80], BF16, tag="wb" + nm)
                nc.vector.tensor_copy(wb[:], wf[:])
                wts.append(wb)
            wta, wtb, wtc = wts
            poolm_f = const.tile([128, GPB], F32)
            nc.sync.dma_start(poolm_f[:], poolm_ap[:])
            poolm = const.tile([128, GPB], BF16)
            nc.vector.tensor_copy(poolm[:], poolm_f[:])
            wr1_f = const.tile([80, HID], F32)
            nc.scalar.dma_start(wr1_f[:], wr1_ap[:])
            wr1 = const.tile([80, HID], F32R)
            nc.vector.tensor_copy(wr1[:], wr1_f[:])
            wr2a_f = const.tile([128, LAT], F32)
            nc.scalar.dma_start(wr2a_f[:], wr2_ap[0:128, :])
            wr2a = const.tile([128, LAT], F32R)
            nc.vector.tensor_copy(wr2a[:], wr2a_f[:])
            wr2b_f = const.tile([128, LAT], F32)
            nc.scalar.dma_start(wr2b_f[:], wr2_ap[128:256, :])
            wr2b = const.tile([128, LAT], F32R)
            nc.vector.tensor_copy(wr2b[:], wr2b_f[:])
            br1a = const.tile([128, 1], F32)
            nc.sync.dma_start(br1a[:], br1_ap[0:128, :])
            br1b = const.tile([128, 1], F32)
            nc.sync.dma_start(br1b[:], br1_ap[128:256, :])
            br2 = const.tile([LAT, 1], F32)
            nc.sync.dma_start(br2[:], br2_ap[:])
            epsb = const.tile([128, 1], F32)
            nc.vector.memset(epsb[:], 1e-4 / 3.0)

            # pooled per-graph features, feature-major [80, 256]
            xfm = const.tile([80, GPC], F32R)

            GRPP = 4   # pairs per group

            def emit_group_stage(g):
                NB8 = GRPP * 2
                glg = stage.tile([24, NB8 * 128], BF16, tag="glg")
                nc.scalar.dma_start(
                    glg[:],
                    bass.AP(tensor=gl_ap.tensor,
                            offset=gl_ap.offset + g * NB8 * 24 * 128,
                            ap=[[128, 24], [24 * 128, NB8], [1, 128]]),
                )
                grg = stage.tile([24, NB8 * 512], BF16, tag="grg")
                nc.sync.dma_start(
                    grg[:],
                    bass.AP(tensor=gr_ap.tensor,
                            offset=gr_ap.offset + g * NB8 * 24 * 512,
                            ap=[[512, 24], [24 * 512, NB8], [1, 512]]),
                )
                return glg, grg

            def emit_geo(p, i, gs):
                # p: global pair idx, i: pair-in-group idx
                glg, grg = gs
                gm = gmp.tile([128, 1024], BF16, tag="gm")
                for h in range(2):
                    b = 2 * i + h
                    pg = psp_geo.tile([128, 512], F32, tag="pg")
                    nc.tensor.matmul(
                        pg[:], glg[:, b * 128:(b + 1) * 128],
                        grg[:, b * 512:(b + 1) * 512], start=True, stop=True)
                    # mask = is_le(d2, 25) * bd
                    nc.vector.scalar_tensor_tensor(
                        gm[:, h * 512:h * 512 + 128], pg[:, 384:512], 25.0,
                        bd[:], AluOpType.is_le, AluOpType.mult)
                    # s3 = sqrt(d2/3 + eps); rs = 1/s3 = sqrt(3)/|r|
                    s3 = work.tile([128, 128], F32, tag="s3")
                    nc.scalar.activation(
                        s3[:], pg[:, 384:512],
                        mybir.ActivationFunctionType.Sqrt,
                        bias=epsb[:], scale=float(1.0 / 3.0))
                    rs = work.tile([128, 128], F32, tag="rs")
                    nc.vector.reciprocal_approx_fast(rs[:], s3[:])
                    # ga = rs * mask
                    ga = work.tile([128, 128], F32, tag="ga")
                    nc.gpsimd.tensor_mul(ga[:], rs[:], gm[:, h * 512:h * 512 + 128])
                    # diff PSUM -> SBUF bf16 (scalar), gm_sh = diff*ga (gpsimd)
                    diffs = work.tile([128, 384], BF16, tag="diffs")
                    nc.scalar.copy(diffs[:], pg[:, 0:384])
                    nc.gpsimd.tensor_mul(
                        bass.AP(tensor=gm.tensor, offset=gm.offset + h * 512 + 128,
                                ap=[[gm.shape[1], 128], [128, 3], [1, 128]]),
                        diffs[:],
                        bass.AP(tensor=ga.tensor, offset=ga.offset,
                                ap=[[ga.shape[1], 128], [0, 3], [1, 128]]))

                # node features bf16 pair tile
                featb = featbp.tile([128, 256], BF16, tag="fb")
                nc.gpsimd.memset(featb[:, 32:80], 0.0)
                nc.gpsimd.memset(featb[:, 160:208], 0.0)
                nc.sync.dma_start(
                    bass.AP(tensor=featb.tensor, offset=featb.offset,
                            ap=[[featb.shape[1], 128], [128, 2], [1, 32]]),
                    bass.AP(tensor=s0_ap.tensor,
                            offset=s0_ap.offset + p * 256 * S_MUL,
                            ap=[[S_MUL, 128], [128 * S_MUL, 2], [1, S_MUL]]),
                )
                return gm, featb

            def emit_agg(gm, featb):
                pa = psp_agg.tile([80, 1024], F32, tag="agg")
                for h in range(2):
                    nc.tensor.matmul(pa[:, h * 512:(h + 1) * 512],
                                     featb[:, h * 128:h * 128 + 80],
                                     gm[:, h * 512:(h + 1) * 512],
                                     start=True, stop=True)
                return pa

            def emit_copies(pa):
                # pair-batched repack, PSUM -> SBUF bf16
                ssa = ssp.tile([80, 256], BF16, tag="ssa")
                ssb = ssp.tile([112, 256], BF16, tag="ssb")
                ssc = ssp.tile([80, 256], BF16, tag="ssc")

                def pap(rows, coloff):
                    return bass.AP(tensor=pa.tensor, offset=pa.offset + coloff,
                                   ap=[[pa.shape[1], rows], [512, 2], [1, 128]])

                def sap(t, rows):
                    return bass.AP(tensor=t.tensor, offset=t.offset,
                                   ap=[[t.shape[1], rows], [128, 2], [1, 128]])

                nc.vector.tensor_copy(sap(ssa, 80), pap(80, 0))
                nc.vector.tensor_copy(sap(ssb, 64), pap(64, 256))
                nc.scalar.copy(ssb[64:112, :], pap(48, 128))
                nc.scalar.copy(sap(ssc, 80), pap(80, 384))
                return ssa, ssb, ssc

            def emit_transform(l, ss):
                ssa, ssb, ssc = ss
                ph = psp_h.tile([128, 168], F32, tag="psh")
                for h in range(2):
                    sl = slice(h * 128, (h + 1) * 128)
                    ps = ph[:, h * 80:(h + 1) * 80]
                    nc.tensor.matmul(ps, ssa[:, sl], wta[:, l, :],
                                     start=True, stop=False)
                    nc.tensor.matmul(ps, ssb[:, sl], wtb[:, l, :],
                                     start=False, stop=False)
                    nc.tensor.matmul(ps, ssc[:, sl], wtc[:, l, :],
                                     start=False, stop=True)
                return ph

            def emit_resid(ph, featb):
                featbn = featbp.tile([128, 256], BF16, tag="fb")
                nc.vector.scalar_tensor_tensor(
                    bass.AP(tensor=featbn.tensor, offset=featbn.offset,
                            ap=[[featbn.shape[1], 128], [128, 2], [1, 80]]),
                    bass.AP(tensor=ph.tensor, offset=ph.offset,
                            ap=[[ph.shape[1], 128], [80, 2], [1, 80]]),
                    0.0,
                    bass.AP(tensor=featb.tensor, offset=featb.offset,
                            ap=[[featb.shape[1], 128], [128, 2], [1, 80]]),
                    AluOpType.max, AluOpType.add)
                return featbn

            NGRP = NPAIR // GRPP     # 8 groups
            for g in range(NGRP):
                gs = emit_group_stage(g)
                st = [emit_geo(g * GRPP + i, i, gs) for i in range(GRPP)]
                for l in range(3):
                    pas = [emit_agg(st[i][0], st[i][1]) for i in range(GRPP)]
                    sss = [emit_copies(pas[i]) for i in range(GRPP)]
                    for i in range(GRPP):
                        ph = emit_transform(l, sss[i])
                        st[i] = (st[i][0], emit_resid(ph, st[i][1]))
                for i in range(GRPP):
                    # pool into a fresh psh-ring tile (cols 0:8), then to xfm
                    p = g * GRPP + i
                    pp = psp_h.tile([128, 168], F32, tag="psh")
                    featb = st[i][1]
                    for h in range(2):
                        nc.tensor.matmul(
                            pp[0:80, h * GPB:(h + 1) * GPB],
                            featb[:, h * 128:h * 128 + 80],
                            poolm[:], start=True, stop=True)
                    nc.vector.tensor_copy(
                        xfm[0:80, p * 2 * GPB:(p + 1) * 2 * GPB],
                        pp[0:80, 0:2 * GPB])

            # --- readout MLP: relu(x @ Wr1 + br1) @ Wr2 + br2, feature-major ---
            t1 = psp_agg.tile([128, 1024], F32, tag="agg")
            t2 = psp_agg.tile([128, 1024], F32, tag="agg")
            ps_h1 = t1[:, 0:GPC]
            ps_h2 = t2[:, 0:GPC]
            nc.tensor.matmul(ps_h1, wr1[:, 0:128], xfm[:], start=True, stop=True)
            nc.tensor.matmul(ps_h2, wr1[:, 128:256], xfm[:], start=True, stop=True)
            hid1 = work.tile([128, GPC], F32R, tag="hid1")
            hid2 = work.tile([128, GPC], F32R, tag="hid2")
            nc.vector.tensor_scalar(hid1[:], ps_h1, br1a[:], 0.0,
                                    AluOpType.add, AluOpType.max)
            nc.vector.tensor_scalar(hid2[:], ps_h2, br1b[:], 0.0,
                                    AluOpType.add, AluOpType.max)
            t3 = psp_agg.tile([128, 1024], F32, tag="agg")
            ps_o = t3[0:LAT, 0:GPC]
            nc.tensor.matmul(ps_o, wr2a[:], hid1[:], start=True, stop=False)
            nc.tensor.matmul(ps_o, wr2b[:], hid2[:], start=False, stop=True)
            outt = work.tile([LAT, GPC], F32, tag="outt")
            nc.vector.tensor_scalar(outt[:], ps_o, br2[:], None,
                                    AluOpType.add)
            nc.sync.dma_start(out_ap[:], outt[:])

    nc.compile()
    return nc


def _split3(x):
    import ml_dtypes
    h = x.astype(ml_dtypes.bfloat16)
    r = x - h.astype(np.float32)
    m = r.astype(ml_dtypes.bfloat16)
    l = (r - m.astype(np.float32)).astype(ml_dtypes.bfloat16)
    return h, m, l


def kernel(pos, emb, W_s2n, W1, W2, W3, W4, Ws, Wv, Wr1, br1, Wr2, br2,
           z, batch, edge_index, num_graphs):
    import ml_dtypes
    pos = np.asarray(pos, dtype=np.float32)
    z = np.asarray(z)
    emb = np.asarray(emb, dtype=np.float32)
    W_s2n = np.asarray(W_s2n, dtype=np.float32)
    W1 = np.asarray(W1, dtype=np.float32); W2 = np.asarray(W2, dtype=np.float32)
    W3 = np.asarray(W3, dtype=np.float32); W4 = np.asarray(W4, dtype=np.float32)
    Ws = np.asarray(Ws, dtype=np.float32); Wv = np.asarray(Wv, dtype=np.float32)
    Wr1 = np.asarray(Wr1, dtype=np.float32); br1 = np.asarray(br1, dtype=np.float32)
    Wr2 = np.asarray(Wr2, dtype=np.float32); br2 = np.asarray(br2, dtype=np.float32)

    # host prep: embedding lookup folded with input linear
    EW = (emb @ W_s2n) * np.float32(1.0 / np.sqrt(S_MUL))     # [100, 32]
    s0 = EW[z].astype(ml_dtypes.bfloat16)                     # [N, 32] bf16

    # fused transform weights with norm constants folded in
    cs = C_SCALAR * np.float32(1.0 / np.sqrt(S_MUL))
    csb = C_SCALAR * np.float32(INV_SQRT3 / np.sqrt(S_MUL))
    cv = C_VECTOR * np.float32(INV_SQRT3 / np.sqrt(V_MUL))
    wta = np.zeros((3, 80, 80), np.float32)
    wtb = np.zeros((3, 112, 80), np.float32)
    wtc = np.zeros((3, 80, 80), np.float32)
    for l in range(3):
        Wa = cs * (W1[l] @ Ws[l])        # [32,32] s_m -> s
        Wb = csb * (W4[l] @ Ws[l])       # [16,32] v_c*sh_c -> s
        Wc = cv * (W2[l] @ Wv[l])        # [32,16] s*sh_c -> v_c
        Wd = cv * (W3[l] @ Wv[l])        # [16,16] v_c_m -> v_c
        # SSa rows: [s_m(0:32) vx_m(32:48) vy_m(48:64) vz_m(64:80)]
        wta[l, 0:32, 0:32] = Wa
        wta[l, 32:48, 32:48] = Wd
        wta[l, 48:64, 48:64] = Wd
        wta[l, 64:80, 64:80] = Wd
        # SSb rows: [s_y(0:32) waste(32:48) vyy(48:64) s_x(64:96) vxx(96:112)]
        wtb[l, 0:32, 48:64] = Wc
        wtb[l, 48:64, 0:32] = Wb
        wtb[l, 64:96, 32:48] = Wc
        wtb[l, 96:112, 0:32] = Wb
        # SSc rows: [s_z(0:32) waste(32:64) vzz(64:80)]
        wtc[l, 0:32, 64:80] = Wc
        wtc[l, 64:80, 0:32] = Wb

    # readout Wr1 with rows permuted to the [s | vx | vy | vz] feature order
    wr1p = np.zeros((80, HID), np.float32)
    wr1p[0:32] = Wr1[0:32]                        # s
    for u in range(V_MUL):
        wr1p[32 + u] = Wr1[S_MUL + 3 * u + 0]     # vx
        wr1p[48 + u] = Wr1[S_MUL + 3 * u + 1]     # vy
        wr1p[64 + u] = Wr1[S_MUL + 3 * u + 2]     # vz

    bdm = np.zeros((128, 128), np.float32)
    for g in range(GPB):
        bdm[g * NA:(g + 1) * NA, g * NA:(g + 1) * NA] = 1.0
    np.fill_diagonal(bdm, 0.0)                    # no self-loops (d2 > 0)
    poolm = np.zeros((128, GPB), np.float32)
    for g in range(GPB):
        poolm[g * NA:(g + 1) * NA, g] = 1.0

    if "nc" not in _CACHE:
        _CACHE["nc"] = _build_program()
    nc = _CACHE["nc"]

    in_maps = []
    for c in range(NCORES):
        psl = pos[c * NPC:(c + 1) * NPC]                       # [8192, 3]
        pb = psl.reshape(NBLK, 128, 3)                         # [64, 128, 3]
        pbt = np.ascontiguousarray(pb.transpose(0, 2, 1))      # [64, 3, 128]
        nrm2 = (pb.astype(np.float64) ** 2).sum(-1).astype(np.float32)
        ph, pm, pl = _split3(pbt)                              # [64, 3, 128] each
        nh, nm_, nl = _split3(nrm2)                            # [64, 128] each
        phf = ph.astype(np.float32); pmf = pm.astype(np.float32)
        plf = pl.astype(np.float32)
        gl = np.zeros((NBLK, 24, 128), ml_dtypes.bfloat16)
        gr = np.zeros((NBLK, 24, 512), ml_dtypes.bfloat16)
        gl[:, 0:3, :] = 1.0
        gr[:, 0, 384:512] = nh
        gr[:, 1, 384:512] = nm_
        gr[:, 2, 384:512] = nl
        for cc in range(3):
            js = slice(cc * 128, (cc + 1) * 128)
            gr[:, 0, js] = ph[:, cc]
            gr[:, 1, js] = pm[:, cc]
            gr[:, 2, js] = pl[:, cc]
            b0 = 3 + 6 * cc
            # rows: (pi coef, diff rhs, d2 rhs)
            gl[:, b0 + 0, :] = ph[:, cc]
            gl[:, b0 + 1, :] = ph[:, cc]
            gl[:, b0 + 2, :] = ph[:, cc]
            gl[:, b0 + 3, :] = pm[:, cc]
            gl[:, b0 + 4, :] = pm[:, cc]
            gl[:, b0 + 5, :] = pl[:, cc]
            for k in (0, 3, 5):
                gr[:, b0 + k, js] = -1.0
            gr[:, b0 + 0, 384:512] = (-2.0 * phf[:, cc]).astype(ml_dtypes.bfloat16)
            gr[:, b0 + 1, 384:512] = (-2.0 * pmf[:, cc]).astype(ml_dtypes.bfloat16)
            gr[:, b0 + 2, 384:512] = (-2.0 * plf[:, cc]).astype(ml_dtypes.bfloat16)
            gr[:, b0 + 3, 384:512] = (-2.0 * phf[:, cc]).astype(ml_dtypes.bfloat16)
            gr[:, b0 + 4, 384:512] = (-2.0 * pmf[:, cc]).astype(ml_dtypes.bfloat16)
            gr[:, b0 + 5, 384:512] = (-2.0 * phf[:, cc]).astype(ml_dtypes.bfloat16)
        gl[:, 21, :] = nh
        gl[:, 22, :] = nm_
        gl[:, 23, :] = nl
        gr[:, 21:24, 384:512] = 1.0
        in_maps.append(dict(
            s0=np.ascontiguousarray(s0[c * NPC:(c + 1) * NPC]),
            gl=gl, gr=gr,
            bd=bdm, wta=wta, wtb=wtb, wtc=wtc, poolm=poolm,
            wr1=wr1p, br1=br1.reshape(HID, 1),
            wr2=Wr2, br2=br2.reshape(LAT, 1),
        ))

    res = run_bass_kernel_spmd(nc, in_maps, core_ids=list(range(NCORES)))
    out = np.empty((B, LAT), np.float32)
    for c in range(NCORES):
        out[c * GPC:(c + 1) * GPC] = res.results[c]["outfm"].T
    return out


# revision 32
# speedup vs baseline: 1.2398x; 1.0392x over previous
import sys

sys.path.insert(0, "/opt/trn_rl_repo")
import numpy as np
import concourse.bass as bass
import concourse.tile as tile
from concourse import bacc, mybir
from concourse.alu_op_type import AluOpType
from concourse.bass_utils import run_bass_kernel_spmd

# Problem constants (nn_EquivGNNEncoder: 2048 graphs x 32 atoms, 3 layers)
B, NA = 2048, 32
N = B * NA                  # 65536 nodes
S_MUL, V_MUL = 32, 16
NCORES = 8
GPC = B // NCORES           # 256 graphs per core
NPC = GPC * NA              # 8192 nodes per core
GPB = 4                     # graphs per block (4*32 = 128 partitions)
NBLK = GPC // GPB           # 64 blocks per core
LAT = 128                   # latent out dim
HID = 256
NPAIR = NBLK // 2           # 32 block-pairs per core

INV_SQRT3 = 1.0 / np.sqrt(3.0)
C_SCALAR = np.float32(1.0 / np.sqrt(48.0))
C_VECTOR = np.float32(np.sqrt(3.0 / 48.0))

F32 = mybir.dt.float32
F32R = mybir.dt.float32r
BF16 = mybir.dt.bfloat16

_CACHE = {}

# node feature column layout: [s(0:32) | vx(32:48) | vy(48:64) | vz(64:80)]
# geometry: ONE bf16 matmul per block -> PSUM [128, 512]:
#   cols 0:384  diff[i, c*128+j] = pos[j,c]-pos[i,c]  (3-way bf16 split, exact
#     to ~2^-24); cols 384:512 d2[i,j] (split products, err ~1e-4)
# gm pair tile [128, 1024], block h at h*512: [mask(128) | shx | shy | shz]
# ps_agg pair [80, 1024] (lhsT = featb 80 cols); repack per pair-layer:
#   ssa[0:80]   <- PA[0:80, mask]
#   ssb[0:64]   <- PA[0:64, shy]   (s_y, vx junk, vyy)
#   ssb[64:112] <- PA[0:48, shx]   (s_x, vxx)
#   ssc[0:80]   <- PA[0:80, shz]   (s_z, junk, vzz)
# transform = 3 matmuls per block (wta 80, wtb 112, wtc 80 rows)


def _build_program():
    nc = bacc.Bacc("TRN2", target_bir_lowering=False, debug=False)

    s0_ap = nc.dram_tensor("s0", [NPC, S_MUL], BF16, kind="ExternalInput").ap()
    gl_ap = nc.dram_tensor("gl", [NBLK, 24, 128], BF16, kind="ExternalInput").ap()
    gr_ap = nc.dram_tensor("gr", [NBLK, 24, 512], BF16, kind="ExternalInput").ap()
    bd_ap = nc.dram_tensor("bd", [128, 128], F32, kind="ExternalInput").ap()
    wta_ap = nc.dram_tensor("wta", [3, 80, 80], F32, kind="ExternalInput").ap()
    wtb_ap = nc.dram_tensor("wtb", [3, 112, 80], F32, kind="ExternalInput").ap()
    wtc_ap = nc.dram_tensor("wtc", [3, 80, 80], F32, kind="ExternalInput").ap()
    wl0_ap = nc.dram_tensor("wl0", [96, 80], F32, kind="ExternalInput").ap()
    poolm_ap = nc.dram_tensor("poolm", [128, GPB], F32, kind="ExternalInput").ap()
    wr1_ap = nc.dram_tensor("wr1", [80, HID], F32, kind="ExternalInput").ap()
    br1_ap = nc.dram_tensor("br1", [HID, 1], F32, kind="ExternalInput").ap()
    wr2_ap = nc.dram_tensor("wr2", [HID, LAT], F32, kind="ExternalInput").ap()
    br2_ap = nc.dram_tensor("br2", [LAT, 1], F32, kind="ExternalInput").ap()
    out_ap = nc.dram_tensor("outfm", [LAT, GPC], F32, kind="ExternalOutput").ap()

    with tile.TileContext(nc) as tc:
        with tc.tile_pool(name="const", bufs=1) as const, \
             tc.tile_pool(name="stage", bufs=3) as stage, \
             tc.tile_pool(name="gmp", bufs=5) as gmp, \
             tc.tile_pool(name="featb", bufs=10, space="SBUF") as featbp, \
             tc.tile_pool(name="ssp", bufs=8) as ssp, \
             tc.tile_pool(name="work", bufs=6) as work, \
             tc.tile_pool(name="psgeo", bufs=2, space="PSUM") as psp_geo, \
             tc.tile_pool(name="psagg", bufs=2, space="PSUM") as psp_agg, \
             tc.tile_pool(name="psh", bufs=2, space="PSUM") as psp_h:

            # --- constants ---
            bd = const.tile([128, 128], F32)
            nc.sync.dma_start(bd[:], bd_ap[:])
            wts = []
            for nm, ap_, rows in (("a", wta_ap, 80), ("b", wtb_ap, 112),
                                  ("c", wtc_ap, 80)):
                wf = const.tile([rows, 3, 80], F32, tag="wf" + nm)
                nc.scalar.dma_start(
                    wf[:],
                    bass.AP(tensor=ap_.tensor, offset=ap_.offset,
                            ap=[[80, rows], [rows * 80, 3], [1, 80]]),
                )
                wb = const.tile([rows, 3, 80], BF16, tag="wb" + nm)
                nc.vector.tensor_copy(wb[:], wf[:])
                wts.append(wb)
            wta, wtb, wtc = wts
            wl0f = const.tile([96, 80], F32, tag="wl0f")
            nc.scalar.dma_start(wl0f[:], wl0_ap[:])
            wl0 = const.tile([96, 80], BF16, tag="wl0")
            nc.vector.tensor_copy(wl0[:], wl0f[:])
            poolm_f = const.tile([128, GPB], F32)
            nc.sync.dma_start(poolm_f[:], poolm_ap[:])
            poolm = const.tile([128, GPB], BF16)
            nc.vector.tensor_copy(poolm[:], poolm_f[:])
            wr1_f = const.tile([80, HID], F32)
            nc.scalar.dma_start(wr1_f[:], wr1_ap[:])
            wr1 = const.tile([80, HID], F32R)
            nc.vector.tensor_copy(wr1[:], wr1_f[:])
            wr2a_f = const.tile([128, LAT], F32)
            nc.scalar.dma_start(wr2a_f[:], wr2_ap[0:128, :])
            wr2a = const.tile([128, LAT], F32R)
            nc.vector.tensor_copy(wr2a[:], wr2a_f[:])
            wr2b_f = const.tile([128, LAT], F32)
            nc.scalar.dma_start(wr2b_f[:], wr2_ap[128:256, :])
            wr2b = const.tile([128, LAT], F32R)
            nc.vector.tensor_copy(wr2b[:], wr2b_f[:])
            br1a = const.tile([128, 1], F32)
            nc.sync.dma_start(br1a[:], br1_ap[0:128, :])
            br1b = const.tile([128, 1], F32)
            nc.sync.dma_start(br1b[:], br1_ap[128:256, :])
            br2 = const.tile([LAT, 1], F32)
            nc.sync.dma_start(br2[:], br2_ap[:])
            epsb = const.tile([128, 1], F32)
            nc.vector.memset(epsb[:], 1e-4 / 3.0)

            # pooled per-graph features, feature-major [80, 256]
            xfm = const.tile([80, GPC], F32R)

            GRPP = 4   # pairs per group

            def emit_group_stage(g):
                NB8 = GRPP * 2
                glg = stage.tile([24, NB8 * 128], BF16, tag="glg")
                nc.scalar.dma_start(
                    glg[:],
                    bass.AP(tensor=gl_ap.tensor,
                            offset=gl_ap.offset + g * NB8 * 24 * 128,
                            ap=[[128, 24], [24 * 128, NB8], [1, 128]]),
                )
                grg = stage.tile([24, NB8 * 512], BF16, tag="grg")
                nc.sync.dma_start(
                    grg[:],
                    bass.AP(tensor=gr_ap.tensor,
                            offset=gr_ap.offset + g * NB8 * 24 * 512,
                            ap=[[512, 24], [24 * 512, NB8], [1, 512]]),
                )
                return glg, grg

            def emit_geo(p, i, gs):
                # p: global pair idx, i: pair-in-group idx
                glg, grg = gs
                gm = gmp.tile([128, 1024], BF16, tag="gm")
                for h in range(2):
                    b = 2 * i + h
                    pg = psp_geo.tile([128, 512], F32, tag="pg")
                    nc.tensor.matmul(
                        pg[:], glg[:, b * 128:(b + 1) * 128],
                        grg[:, b * 512:(b + 1) * 512], start=True, stop=True)
                    # mask = is_le(d2, 25) * bd
                    nc.vector.scalar_tensor_tensor(
                        gm[:, h * 512:h * 512 + 128], pg[:, 384:512], 25.0,
                        bd[:], AluOpType.is_le, AluOpType.mult)
                    # s3 = sqrt(d2/3 + eps); rs = 1/s3 = sqrt(3)/|r|
                    s3 = work.tile([128, 128], F32, tag="s3")
                    nc.scalar.activation(
                        s3[:], pg[:, 384:512],
                        mybir.ActivationFunctionType.Sqrt,
                        bias=epsb[:], scale=float(1.0 / 3.0))
                    rs = work.tile([128, 128], F32, tag="rs")
                    nc.vector.reciprocal_approx_fast(rs[:], s3[:])
                    # ga = rs * mask
                    ga = work.tile([128, 128], F32, tag="ga")
                    nc.gpsimd.tensor_mul(ga[:], rs[:], gm[:, h * 512:h * 512 + 128])
                    # diff PSUM -> SBUF bf16 (scalar), gm_sh = diff*ga (gpsimd)
                    diffs = work.tile([128, 384], BF16, tag="diffs")
                    nc.scalar.copy(diffs[:], pg[:, 0:384])
                    nc.gpsimd.tensor_mul(
                        bass.AP(tensor=gm.tensor, offset=gm.offset + h * 512 + 128,
                                ap=[[gm.shape[1], 128], [128, 3], [1, 128]]),
                        diffs[:],
                        bass.AP(tensor=ga.tensor, offset=ga.offset,
                                ap=[[ga.shape[1], 128], [0, 3], [1, 128]]))

                # node features bf16 pair tile; v cols zeroed (L0 resid adds them)
                featb = featbp.tile([128, 256], BF16, tag="fb")
                nc.gpsimd.memset(featb[:, 32:80], 0.0)
                nc.gpsimd.memset(featb[:, 160:208], 0.0)
                nc.sync.dma_start(
                    bass.AP(tensor=featb.tensor, offset=featb.offset,
                            ap=[[featb.shape[1], 128], [128, 2], [1, 32]]),
                    bass.AP(tensor=s0_ap.tensor,
                            offset=s0_ap.offset + p * 256 * S_MUL,
                            ap=[[S_MUL, 128], [128 * S_MUL, 2], [1, S_MUL]]),
                )
                return gm, featb

            def emit_agg(l, gm, featb):
                pa = psp_agg.tile([80, 1024], F32, tag="agg")
                w = 32 if l == 0 else 80
                for h in range(2):
                    nc.tensor.matmul(pa[0:w, h * 512:(h + 1) * 512],
                                     featb[:, h * 128:h * 128 + w],
                                     gm[:, h * 512:(h + 1) * 512],
                                     start=True, stop=True)
                return pa

            def emit_copies(l, pa):
                # pair-batched repack, PSUM -> SBUF bf16
                def pap(rows, coloff):
                    return bass.AP(tensor=pa.tensor, offset=pa.offset + coloff,
                                   ap=[[pa.shape[1], rows], [512, 2], [1, 128]])

                def sap(t, rows):
                    return bass.AP(tensor=t.tensor, offset=t.offset,
                                   ap=[[t.shape[1], rows], [128, 2], [1, 128]])

                if l == 0:
                    # only s-features exist: ssa0 = s_m; sxyz = [s_x, s_y, s_z]
                    ssa = ssp.tile([32, 256], BF16, tag="sa0")
                    sxyz = ssp.tile([96, 256], BF16, tag="sxyz")
                    nc.vector.tensor_copy(sap(ssa, 32), pap(32, 0))
                    nc.vector.tensor_copy(sap(sxyz, 32), pap(32, 128))
                    nc.scalar.copy(sxyz[32:64, :], pap(32, 256))
                    nc.scalar.copy(sxyz[64:96, :], pap(32, 384))
                    return ssa, sxyz, None
                ssa = ssp.tile([80, 256], BF16, tag="ssa")
                ssb = ssp.tile([112, 256], BF16, tag="ssb")
                ssc = ssp.tile([80, 256], BF16, tag="ssc")
                nc.vector.tensor_copy(sap(ssa, 80), pap(80, 0))
                nc.vector.tensor_copy(sap(ssb, 64), pap(64, 256))
                nc.scalar.copy(ssb[64:112, :], pap(48, 128))
                nc.scalar.copy(sap(ssc, 80), pap(80, 384))
                return ssa, ssb, ssc

            def emit_transform(l, ss):
                ph = psp_h.tile([128, 168], F32, tag="psh")
                if l == 0:
                    ssa, sxyz, _ = ss
                    for h in range(2):
                        sl = slice(h * 128, (h + 1) * 128)
                        ps = ph[:, h * 80:(h + 1) * 80]
                        nc.tensor.matmul(ps, ssa[:, sl], wta[0:32, 0, :],
                                         start=True, stop=False)
                        nc.tensor.matmul(ps, sxyz[:, sl], wl0[:],
                                         start=False, stop=True)
                    return ph
                ssa, ssb, ssc = ss
                for h in range(2):
                    sl = slice(h * 128, (h + 1) * 128)
                    ps = ph[:, h * 80:(h + 1) * 80]
                    nc.tensor.matmul(ps, ssa[:, sl], wta[:, l, :],
                                     start=True, stop=False)
                    nc.tensor.matmul(ps, ssb[:, sl], wtb[:, l, :],
                                     start=False, stop=False)
                    nc.tensor.matmul(ps, ssc[:, sl], wtc[:, l, :],
                                     start=False, stop=True)
                return ph

            def emit_resid(ph, featb):
                featbn = featbp.tile([128, 256], BF16, tag="fb")
                nc.vector.scalar_tensor_tensor(
                    bass.AP(tensor=featbn.tensor, offset=featbn.offset,
                            ap=[[featbn.shape[1], 128], [128, 2], [1, 80]]),
                    bass.AP(tensor=ph.tensor, offset=ph.offset,
                            ap=[[ph.shape[1], 128], [80, 2], [1, 80]]),
                    0.0,
                    bass.AP(tensor=featb.tensor, offset=featb.offset,
                            ap=[[featb.shape[1], 128], [128, 2], [1, 80]]),
                    AluOpType.max, AluOpType.add)
                return featbn

            NGRP = NPAIR // GRPP     # 8 groups
            for g in range(NGRP):
                gs = emit_group_stage(g)
                st = [emit_geo(g * GRPP + i, i, gs) for i in range(GRPP)]
                for l in range(3):
                    pas = [emit_agg(l, st[i][0], st[i][1]) for i in range(GRPP)]
                    sss = [emit_copies(l, pas[i]) for i in range(GRPP)]
                    for i in range(GRPP):
                        ph = emit_transform(l, sss[i])
                        st[i] = (st[i][0], emit_resid(ph, st[i][1]))
                # pool the whole group into one psh-ring tile, then to xfm
                pp = psp_h.tile([128, 168], F32, tag="psh")
                for i in range(GRPP):
                    featb = st[i][1]
                    for h in range(2):
                        nc.tensor.matmul(
                            pp[0:80, (2 * i + h) * GPB:(2 * i + h + 1) * GPB],
                            featb[:, h * 128:h * 128 + 80],
                            poolm[:], start=True, stop=True)
                nc.vector.tensor_copy(
                    xfm[0:80, g * GRPP * 2 * GPB:(g + 1) * GRPP * 2 * GPB],
                    pp[0:80, 0:GRPP * 2 * GPB])

            # --- readout MLP: relu(x @ Wr1 + br1) @ Wr2 + br2, feature-major ---
            t1 = psp_agg.tile([128, 1024], F32, tag="agg")
            t2 = psp_agg.tile([128, 1024], F32, tag="agg")
            ps_h1 = t1[:, 0:GPC]
            ps_h2 = t2[:, 0:GPC]
            nc.tensor.matmul(ps_h1, wr1[:, 0:128], xfm[:], start=True, stop=True)
            nc.tensor.matmul(ps_h2, wr1[:, 128:256], xfm[:], start=True, stop=True)
            hid1 = work.tile([128, GPC], F32R, tag="hid1")
            hid2 = work.tile([128, GPC], F32R, tag="hid2")
            nc.vector.tensor_scalar(hid1[:], ps_h1, br1a[:], 0.0,
                                    AluOpType.add, AluOpType.max)
            nc.vector.tensor_scalar(hid2[:], ps_h2, br1b[:], 0.0,
                                    AluOpType.add, AluOpType.max)
            t3 = psp_agg.tile([128, 1024], F32, tag="agg")
            ps_o = t3[0:LAT, 0:GPC]
            nc.tensor.matmul(ps_o, wr2a[:], hid1[:], start=True, stop=False)
            nc.tensor.matmul(ps_o, wr2b[:], hid2[:], start=False, stop=True)
            outt = work.tile([LAT, GPC], F32, tag="outt")
            nc.vector.tensor_scalar(outt[:], ps_o, br2[:], None,
                                    AluOpType.add)
            nc.sync.dma_start(out_ap[:], outt[:])

    nc.compile()
    return nc


def _split3(x):
    import ml_dtypes
    h = x.astype(ml_dtypes.bfloat16)
    r = x - h.astype(np.float32)
    m = r.astype(ml_dtypes.bfloat16)
    l = (r - m.astype(np.float32)).astype(ml_dtypes.bfloat16)
    return h, m, l


def kernel(pos, emb, W_s2n, W1, W2, W3, W4, Ws, Wv, Wr1, br1, Wr2, br2,
           z, batch, edge_index, num_graphs):
    import ml_dtypes
    pos = np.asarray(pos, dtype=np.float32)
    z = np.asarray(z)
    emb = np.asarray(emb, dtype=np.float32)
    W_s2n = np.asarray(W_s2n, dtype=np.float32)
    W1 = np.asarray(W1, dtype=np.float32); W2 = np.asarray(W2, dtype=np.float32)
    W3 = np.asarray(W3, dtype=np.float32); W4 = np.asarray(W4, dtype=np.float32)
    Ws = np.asarray(Ws, dtype=np.float32); Wv = np.asarray(Wv, dtype=np.float32)
    Wr1 = np.asarray(Wr1, dtype=np.float32); br1 = np.asarray(br1, dtype=np.float32)
    Wr2 = np.asarray(Wr2, dtype=np.float32); br2 = np.asarray(br2, dtype=np.float32)

    # host prep: embedding lookup folded with input linear
    EW = (emb @ W_s2n) * np.float32(1.0 / np.sqrt(S_MUL))     # [100, 32]
    s0 = EW[z].astype(ml_dtypes.bfloat16)                     # [N, 32] bf16

    # fused transform weights with norm constants folded in
    cs = C_SCALAR * np.float32(1.0 / np.sqrt(S_MUL))
    csb = C_SCALAR * np.float32(INV_SQRT3 / np.sqrt(S_MUL))
    cv = C_VECTOR * np.float32(INV_SQRT3 / np.sqrt(V_MUL))
    wta = np.zeros((3, 80, 80), np.float32)
    wtb = np.zeros((3, 112, 80), np.float32)
    wtc = np.zeros((3, 80, 80), np.float32)
    for l in range(3):
        Wa = cs * (W1[l] @ Ws[l])        # [32,32] s_m -> s
        Wb = csb * (W4[l] @ Ws[l])       # [16,32] v_c*sh_c -> s
        Wc = cv * (W2[l] @ Wv[l])        # [32,16] s*sh_c -> v_c
        Wd = cv * (W3[l] @ Wv[l])        # [16,16] v_c_m -> v_c
        # SSa rows: [s_m(0:32) vx_m(32:48) vy_m(48:64) vz_m(64:80)]
        wta[l, 0:32, 0:32] = Wa
        wta[l, 32:48, 32:48] = Wd
        wta[l, 48:64, 48:64] = Wd
        wta[l, 64:80, 64:80] = Wd
        # SSb rows: [s_y(0:32) waste(32:48) vyy(48:64) s_x(64:96) vxx(96:112)]
        wtb[l, 0:32, 48:64] = Wc
        wtb[l, 48:64, 0:32] = Wb
        wtb[l, 64:96, 32:48] = Wc
        wtb[l, 96:112, 0:32] = Wb
        # SSc rows: [s_z(0:32) waste(32:64) vzz(64:80)]
        wtc[l, 0:32, 64:80] = Wc
        wtc[l, 64:80, 0:32] = Wb

    # L0-only transform: input rows [s_x(0:32) s_y(32:64) s_z(64:96)]
    wl0 = np.zeros((96, 80), np.float32)
    wl0[0:32] = wtb[0, 64:96]      # s_x -> v_x
    wl0[32:64] = wtb[0, 0:32]      # s_y -> v_y
    wl0[64:96] = wtc[0, 0:32]      # s_z -> v_z

    # readout Wr1 with rows permuted to the [s | vx | vy | vz] feature order
    wr1p = np.zeros((80, HID), np.float32)
    wr1p[0:32] = Wr1[0:32]                        # s
    for u in range(V_MUL):
        wr1p[32 + u] = Wr1[S_MUL + 3 * u + 0]     # vx
        wr1p[48 + u] = Wr1[S_MUL + 3 * u + 1]     # vy
        wr1p[64 + u] = Wr1[S_MUL + 3 * u + 2]     # vz

    bdm = np.zeros((128, 128), np.float32)
    for g in range(GPB):
        bdm[g * NA:(g + 1) * NA, g * NA:(g + 1) * NA] = 1.0
    np.fill_diagonal(bdm, 0.0)                    # no self-loops (d2 > 0)
    poolm = np.zeros((128, GPB), np.float32)
    for g in range(GPB):
        poolm[g * NA:(g + 1) * NA, g] = 1.0

    if "nc" not in _CACHE:
        _CACHE["nc"] = _build_program()
    nc = _CACHE["nc"]

    in_maps = []
    for c in range(NCORES):
        psl = pos[c * NPC:(c + 1) * NPC]                       # [8192, 3]
        pb = psl.reshape(NBLK, 128, 3)                         # [64, 128, 3]
        pbt = np.ascontiguousarray(pb.transpose(0, 2, 1))      # [64, 3, 128]
        nrm2 = (pb.astype(np.float64) ** 2).sum(-1).astype(np.float32)
        ph, pm, pl = _split3(pbt)                              # [64, 3, 128] each
        nh, nm_, nl = _split3(nrm2)                            # [64, 128] each
        phf = ph.astype(np.float32); pmf = pm.astype(np.float32)
        plf = pl.astype(np.float32)
        gl = np.zeros((NBLK, 24, 128), ml_dtypes.bfloat16)
        gr = np.zeros((NBLK, 24, 512), ml_dtypes.bfloat16)
        gl[:, 0:3, :] = 1.0
        gr[:, 0, 384:512] = nh
        gr[:, 1, 384:512] = nm_
        gr[:, 2, 384:512] = nl
        for cc in range(3):
            js = slice(cc * 128, (cc + 1) * 128)
            gr[:, 0, js] = ph[:, cc]
            gr[:, 1, js] = pm[:, cc]
            gr[:, 2, js] = pl[:, cc]
            b0 = 3 + 6 * cc
            # rows: (pi coef, diff rhs, d2 rhs)
            gl[:, b0 + 0, :] = ph[:, cc]
            gl[:, b0 + 1, :] = ph[:, cc]
            gl[:, b0 + 2, :] = ph[:, cc]
            gl[:, b0 + 3, :] = pm[:, cc]
            gl[:, b0 + 4, :] = pm[:, cc]
            gl[:, b0 + 5, :] = pl[:, cc]
            for k in (0, 3, 5):
                gr[:, b0 + k, js] = -1.0
            gr[:, b0 + 0, 384:512] = (-2.0 * phf[:, cc]).astype(ml_dtypes.bfloat16)
            gr[:, b0 + 1, 384:512] = (-2.0 * pmf[:, cc]).astype(ml_dtypes.bfloat16)
            gr[:, b0 + 2, 384:512] = (-2.0 * plf[:, cc]).astype(ml_dtypes.bfloat16)
            gr[:, b0 + 3, 384:512] = (-2.0 * phf[:, cc]).astype(ml_dtypes.bfloat16)
            gr[:, b0 + 4, 384:512] = (-2.0 * pmf[:, cc]).astype(ml_dtypes.bfloat16)
            gr[:, b0 + 5, 384:512] = (-2.0 * phf[:, cc]).astype(ml_dtypes.bfloat16)
        gl[:, 21, :] = nh
        gl[:, 22, :] = nm_
        gl[:, 23, :] = nl
        gr[:, 21:24, 384:512] = 1.0
        in_maps.append(dict(
            s0=np.ascontiguousarray(s0[c * NPC:(c + 1) * NPC]),
            gl=gl, gr=gr,
            bd=bdm, wta=wta, wtb=wtb, wtc=wtc, wl0=wl0, poolm=poolm,
            wr1=wr1p, br1=br1.reshape(HID, 1),
            wr2=Wr2, br2=br2.reshape(LAT, 1),
        ))

    res = run_bass_kernel_spmd(nc, in_maps, core_ids=list(range(NCORES)))
    out = np.empty((B, LAT), np.float32)
    for c in range(NCORES):
        out[c * GPC:(c + 1) * GPC] = res.results[c]["outfm"].T
    return out


# revision 34
# speedup vs baseline: 1.2572x; 1.0140x over previous
import sys

sys.path.insert(0, "/opt/trn_rl_repo")
import numpy as np
import concourse.bass as bass
import concourse.tile as tile
from concourse import bacc, mybir
from concourse.alu_op_type import AluOpType
from concourse.bass_utils import run_bass_kernel_spmd

# Problem constants (nn_EquivGNNEncoder: 2048 graphs x 32 atoms, 3 layers)
B, NA = 2048, 32
N = B * NA                  # 65536 nodes
S_MUL, V_MUL = 32, 16
NCORES = 8
GPC = B // NCORES           # 256 graphs per core
NPC = GPC * NA              # 8192 nodes per core
GPB = 4                     # graphs per block (4*32 = 128 partitions)
NBLK = GPC // GPB           # 64 blocks per core
LAT = 128                   # latent out dim
HID = 256
NPAIR = NBLK // 2           # 32 block-pairs per core

INV_SQRT3 = 1.0 / np.sqrt(3.0)
C_SCALAR = np.float32(1.0 / np.sqrt(48.0))
C_VECTOR = np.float32(np.sqrt(3.0 / 48.0))

F32 = mybir.dt.float32
F32R = mybir.dt.float32r
BF16 = mybir.dt.bfloat16

_CACHE = {}

# node feature column layout: [s(0:32) | vx(32:48) | vy(48:64) | vz(64:80)]
# geometry: ONE bf16 matmul per block -> PSUM [128, 512]:
#   cols 0:384  diff[i, c*128+j] = pos[j,c]-pos[i,c]  (3-way bf16 split, exact
#     to ~2^-24); cols 384:512 d2[i,j] (split products, err ~1e-4)
# gm pair tile [128, 1024], block h at h*512: [mask(128) | shx | shy | shz]
# ps_agg pair [80, 1024] (lhsT = featb 80 cols); repack per pair-layer:
#   ssa[0:80]   <- PA[0:80, mask]
#   ssb[0:64]   <- PA[0:64, shy]   (s_y, vx junk, vyy)
#   ssb[64:112] <- PA[0:48, shx]   (s_x, vxx)
#   ssc[0:80]   <- PA[0:80, shz]   (s_z, junk, vzz)
# transform = 3 matmuls per block (wta 80, wtb 112, wtc 80 rows)


def _build_program():
    nc = bacc.Bacc("TRN2", target_bir_lowering=False, debug=False)

    s0_ap = nc.dram_tensor("s0", [NPC, S_MUL], BF16, kind="ExternalInput").ap()
    gl_ap = nc.dram_tensor("gl", [NBLK, 24, 128], BF16, kind="ExternalInput").ap()
    gr_ap = nc.dram_tensor("gr", [NBLK, 24, 512], BF16, kind="ExternalInput").ap()
    bd_ap = nc.dram_tensor("bd", [128, 128], F32, kind="ExternalInput").ap()
    wta_ap = nc.dram_tensor("wta", [3, 80, 80], F32, kind="ExternalInput").ap()
    wtb_ap = nc.dram_tensor("wtb", [3, 112, 80], F32, kind="ExternalInput").ap()
    wtc_ap = nc.dram_tensor("wtc", [3, 80, 80], F32, kind="ExternalInput").ap()
    wl0_ap = nc.dram_tensor("wl0", [96, 80], F32, kind="ExternalInput").ap()
    poolm_ap = nc.dram_tensor("poolm", [128, GPB], F32, kind="ExternalInput").ap()
    wr1_ap = nc.dram_tensor("wr1", [80, HID], F32, kind="ExternalInput").ap()
    br1_ap = nc.dram_tensor("br1", [HID, 1], F32, kind="ExternalInput").ap()
    wr2_ap = nc.dram_tensor("wr2", [HID, LAT], F32, kind="ExternalInput").ap()
    br2_ap = nc.dram_tensor("br2", [LAT, 1], F32, kind="ExternalInput").ap()
    out_ap = nc.dram_tensor("outfm", [LAT, GPC], F32, kind="ExternalOutput").ap()

    with tile.TileContext(nc) as tc:
        with tc.tile_pool(name="const", bufs=1) as const, \
             tc.tile_pool(name="stage", bufs=3) as stage, \
             tc.tile_pool(name="gmp", bufs=10) as gmp, \
             tc.tile_pool(name="featb", bufs=18, space="SBUF") as featbp, \
             tc.tile_pool(name="ssp", bufs=10) as ssp, \
             tc.tile_pool(name="work", bufs=10) as work, \
             tc.tile_pool(name="psgeo", bufs=2, space="PSUM") as psp_geo, \
             tc.tile_pool(name="psagg", bufs=2, space="PSUM") as psp_agg, \
             tc.tile_pool(name="psh", bufs=2, space="PSUM") as psp_h:

            # --- constants ---
            bd = const.tile([128, 128], F32)
            nc.sync.dma_start(bd[:], bd_ap[:])
            wts = []
            for nm, ap_, rows in (("a", wta_ap, 80), ("b", wtb_ap, 112),
                                  ("c", wtc_ap, 80)):
                wf = const.tile([rows, 3, 80], F32, tag="wf" + nm)
                nc.scalar.dma_start(
                    wf[:],
                    bass.AP(tensor=ap_.tensor, offset=ap_.offset,
                            ap=[[80, rows], [rows * 80, 3], [1, 80]]),
                )
                wb = const.tile([rows, 3, 80], BF16, tag="wb" + nm)
                nc.vector.tensor_copy(wb[:], wf[:])
                wts.append(wb)
            wta, wtb, wtc = wts
            wl0f = const.tile([96, 80], F32, tag="wl0f")
            nc.scalar.dma_start(wl0f[:], wl0_ap[:])
            wl0 = const.tile([96, 80], BF16, tag="wl0")
            nc.vector.tensor_copy(wl0[:], wl0f[:])
            poolm_f = const.tile([128, GPB], F32)
            nc.sync.dma_start(poolm_f[:], poolm_ap[:])
            poolm = const.tile([128, GPB], BF16)
            nc.vector.tensor_copy(poolm[:], poolm_f[:])
            wr1_f = const.tile([80, HID], F32)
            nc.scalar.dma_start(wr1_f[:], wr1_ap[:])
            wr1 = const.tile([80, HID], F32R)
            nc.vector.tensor_copy(wr1[:], wr1_f[:])
            wr2a_f = const.tile([128, LAT], F32)
            nc.scalar.dma_start(wr2a_f[:], wr2_ap[0:128, :])
            wr2a = const.tile([128, LAT], F32R)
            nc.vector.tensor_copy(wr2a[:], wr2a_f[:])
            wr2b_f = const.tile([128, LAT], F32)
            nc.scalar.dma_start(wr2b_f[:], wr2_ap[128:256, :])
            wr2b = const.tile([128, LAT], F32R)
            nc.vector.tensor_copy(wr2b[:], wr2b_f[:])
            br1a = const.tile([128, 1], F32)
            nc.sync.dma_start(br1a[:], br1_ap[0:128, :])
            br1b = const.tile([128, 1], F32)
            nc.sync.dma_start(br1b[:], br1_ap[128:256, :])
            br2 = const.tile([LAT, 1], F32)
            nc.sync.dma_start(br2[:], br2_ap[:])
            epsb = const.tile([128, 1], F32)
            nc.vector.memset(epsb[:], 1e-4 / 3.0)

            # pooled per-graph features, feature-major [80, 256]
            xfm = const.tile([80, GPC], F32R)

            GRPP = 8   # pairs per group

            def emit_group_stage(g):
                NB8 = GRPP * 2
                glg = stage.tile([24, NB8 * 128], BF16, tag="glg")
                nc.scalar.dma_start(
                    glg[:],
                    bass.AP(tensor=gl_ap.tensor,
                            offset=gl_ap.offset + g * NB8 * 24 * 128,
                            ap=[[128, 24], [24 * 128, NB8], [1, 128]]),
                )
                grg = stage.tile([24, NB8 * 512], BF16, tag="grg")
                nc.sync.dma_start(
                    grg[:],
                    bass.AP(tensor=gr_ap.tensor,
                            offset=gr_ap.offset + g * NB8 * 24 * 512,
                            ap=[[512, 24], [24 * 512, NB8], [1, 512]]),
                )
                return glg, grg

            def emit_geo(p, i, gs):
                # p: global pair idx, i: pair-in-group idx
                glg, grg = gs
                gm = gmp.tile([128, 1024], BF16, tag="gm")
                for h in range(2):
                    b = 2 * i + h
                    pg = psp_geo.tile([128, 512], F32, tag="pg")
                    nc.tensor.matmul(
                        pg[:], glg[:, b * 128:(b + 1) * 128],
                        grg[:, b * 512:(b + 1) * 512], start=True, stop=True)
                    # mask = is_le(d2, 25) * bd
                    nc.vector.scalar_tensor_tensor(
                        gm[:, h * 512:h * 512 + 128], pg[:, 384:512], 25.0,
                        bd[:], AluOpType.is_le, AluOpType.mult)
                    # s3 = sqrt(d2/3 + eps); rs = 1/s3 = sqrt(3)/|r|
                    s3 = work.tile([128, 128], F32, tag="s3")
                    nc.scalar.activation(
                        s3[:], pg[:, 384:512],
                        mybir.ActivationFunctionType.Sqrt,
                        bias=epsb[:], scale=float(1.0 / 3.0))
                    rs = work.tile([128, 128], F32, tag="rs")
                    nc.vector.reciprocal_approx_fast(rs[:], s3[:])
                    # ga = rs * mask
                    ga = work.tile([128, 128], F32, tag="ga")
                    nc.gpsimd.tensor_mul(ga[:], rs[:], gm[:, h * 512:h * 512 + 128])
                    # diff PSUM -> SBUF bf16 (scalar), gm_sh = diff*ga (gpsimd)
                    diffs = work.tile([128, 384], BF16, tag="diffs")
                    nc.scalar.copy(diffs[:], pg[:, 0:384])
                    nc.gpsimd.tensor_mul(
                        bass.AP(tensor=gm.tensor, offset=gm.offset + h * 512 + 128,
                                ap=[[gm.shape[1], 128], [128, 3], [1, 128]]),
                        diffs[:],
                        bass.AP(tensor=ga.tensor, offset=ga.offset,
                                ap=[[ga.shape[1], 128], [0, 3], [1, 128]]))

                # node features bf16 pair tile; v cols zeroed (L0 resid adds them)
                featb = featbp.tile([128, 256], BF16, tag="fb")
                nc.gpsimd.memset(featb[:, 32:80], 0.0)
                nc.gpsimd.memset(featb[:, 160:208], 0.0)
                nc.sync.dma_start(
                    bass.AP(tensor=featb.tensor, offset=featb.offset,
                            ap=[[featb.shape[1], 128], [128, 2], [1, 32]]),
                    bass.AP(tensor=s0_ap.tensor,
                            offset=s0_ap.offset + p * 256 * S_MUL,
                            ap=[[S_MUL, 128], [128 * S_MUL, 2], [1, S_MUL]]),
                )
                return gm, featb

            def emit_agg(l, gm, featb):
                pa = psp_agg.tile([80, 1024], F32, tag="agg")
                w = 32 if l == 0 else 80
                for h in range(2):
                    nc.tensor.matmul(pa[0:w, h * 512:(h + 1) * 512],
                                     featb[:, h * 128:h * 128 + w],
                                     gm[:, h * 512:(h + 1) * 512],
                                     start=True, stop=True)
                return pa

            def emit_copies(l, pa):
                # pair-batched repack, PSUM -> SBUF bf16
                def pap(rows, coloff):
                    return bass.AP(tensor=pa.tensor, offset=pa.offset + coloff,
                                   ap=[[pa.shape[1], rows], [512, 2], [1, 128]])

                def sap(t, rows):
                    return bass.AP(tensor=t.tensor, offset=t.offset,
                                   ap=[[t.shape[1], rows], [128, 2], [1, 128]])

                if l == 0:
                    # only s-features exist: ssa0 = s_m; sxyz = [s_x, s_y, s_z]
                    ssa = ssp.tile([32, 256], BF16, tag="sa0")
                    sxyz = ssp.tile([96, 256], BF16, tag="sxyz")
                    nc.vector.tensor_copy(sap(ssa, 32), pap(32, 0))
                    nc.vector.tensor_copy(sap(sxyz, 32), pap(32, 128))
                    nc.scalar.copy(sxyz[32:64, :], pap(32, 256))
                    nc.scalar.copy(sxyz[64:96, :], pap(32, 384))
                    return ssa, sxyz, None
                ssa = ssp.tile([80, 256], BF16, tag="ssa")
                ssb = ssp.tile([112, 256], BF16, tag="ssb")
                ssc = ssp.tile([80, 256], BF16, tag="ssc")
                nc.vector.tensor_copy(sap(ssa, 80), pap(80, 0))
                nc.vector.tensor_copy(sap(ssb, 64), pap(64, 256))
                nc.scalar.copy(ssb[64:112, :], pap(48, 128))
                nc.scalar.copy(sap(ssc, 80), pap(80, 384))
                return ssa, ssb, ssc

            def emit_transform(l, ss):
                ph = psp_h.tile([128, 168], F32, tag="psh")
                if l == 0:
                    ssa, sxyz, _ = ss
                    for h in range(2):
                        sl = slice(h * 128, (h + 1) * 128)
                        ps = ph[:, h * 80:(h + 1) * 80]
                        nc.tensor.matmul(ps, ssa[:, sl], wta[0:32, 0, :],
                                         start=True, stop=False)
                        nc.tensor.matmul(ps, sxyz[:, sl], wl0[:],
                                         start=False, stop=True)
                    return ph
                ssa, ssb, ssc = ss
                for h in range(2):
                    sl = slice(h * 128, (h + 1) * 128)
                    ps = ph[:, h * 80:(h + 1) * 80]
                    nc.tensor.matmul(ps, ssa[:, sl], wta[:, l, :],
                                     start=True, stop=False)
                    nc.tensor.matmul(ps, ssb[:, sl], wtb[:, l, :],
                                     start=False, stop=False)
                    nc.tensor.matmul(ps, ssc[:, sl], wtc[:, l, :],
                                     start=False, stop=True)
                return ph

            def emit_resid(ph, featb):
                featbn = featbp.tile([128, 256], BF16, tag="fb")
                nc.vector.scalar_tensor_tensor(
                    bass.AP(tensor=featbn.tensor, offset=featbn.offset,
                            ap=[[featbn.shape[1], 128], [128, 2], [1, 80]]),
                    bass.AP(tensor=ph.tensor, offset=ph.offset,
                            ap=[[ph.shape[1], 128], [80, 2], [1, 80]]),
                    0.0,
                    bass.AP(tensor=featb.tensor, offset=featb.offset,
                            ap=[[featb.shape[1], 128], [128, 2], [1, 80]]),
                    AluOpType.max, AluOpType.add)
                return featbn

            NGRP = NPAIR // GRPP     # 8 groups
            for g in range(NGRP):
                gs = emit_group_stage(g)
                st = [emit_geo(g * GRPP + i, i, gs) for i in range(GRPP)]
                for l in range(3):
                    pas = [emit_agg(l, st[i][0], st[i][1]) for i in range(GRPP)]
                    sss = [emit_copies(l, pas[i]) for i in range(GRPP)]
                    for i in range(GRPP):
                        ph = emit_transform(l, sss[i])
                        st[i] = (st[i][0], emit_resid(ph, st[i][1]))
                # pool the whole group into one psh-ring tile, then to xfm
                pp = psp_h.tile([128, 168], F32, tag="psh")
                for i in range(GRPP):
                    featb = st[i][1]
                    for h in range(2):
                        nc.tensor.matmul(
                            pp[0:80, (2 * i + h) * GPB:(2 * i + h + 1) * GPB],
                            featb[:, h * 128:h * 128 + 80],
                            poolm[:], start=True, stop=True)
                nc.vector.tensor_copy(
                    xfm[0:80, g * GRPP * 2 * GPB:(g + 1) * GRPP * 2 * GPB],
                    pp[0:80, 0:GRPP * 2 * GPB])

            # --- readout MLP: relu(x @ Wr1 + br1) @ Wr2 + br2, feature-major ---
            t1 = psp_agg.tile([128, 1024], F32, tag="agg")
            t2 = psp_agg.tile([128, 1024], F32, tag="agg")
            ps_h1 = t1[:, 0:GPC]
            ps_h2 = t2[:, 0:GPC]
            nc.tensor.matmul(ps_h1, wr1[:, 0:128], xfm[:], start=True, stop=True)
            nc.tensor.matmul(ps_h2, wr1[:, 128:256], xfm[:], start=True, stop=True)
            hid1 = work.tile([128, GPC], F32R, tag="hid1")
            hid2 = work.tile([128, GPC], F32R, tag="hid2")
            nc.vector.tensor_scalar(hid1[:], ps_h1, br1a[:], 0.0,
                                    AluOpType.add, AluOpType.max)
            nc.vector.tensor_scalar(hid2[:], ps_h2, br1b[:], 0.0,
                                    AluOpType.add, AluOpType.max)
            t3 = psp_agg.tile([128, 1024], F32, tag="agg")
            ps_o = t3[0:LAT, 0:GPC]
            nc.tensor.matmul(ps_o, wr2a[:], hid1[:], start=True, stop=False)
            nc.tensor.matmul(ps_o, wr2b[:], hid2[:], start=False, stop=True)
            outt = work.tile([LAT, GPC], F32, tag="outt")
            nc.vector.tensor_scalar(outt[:], ps_o, br2[:], None,
                                    AluOpType.add)
            nc.sync.dma_start(out_ap[:], outt[:])

    nc.compile()
    return nc


def _split3(x):
    import ml_dtypes
    h = x.astype(ml_dtypes.bfloat16)
    r = x - h.astype(np.float32)
    m = r.astype(ml_dtypes.bfloat16)
    l = (r - m.astype(np.float32)).astype(ml_dtypes.bfloat16)
    return h, m, l


def kernel(pos, emb, W_s2n, W1, W2, W3, W4, Ws, Wv, Wr1, br1, Wr2, br2,
           z, batch, edge_index, num_graphs):
    import ml_dtypes
    pos = np.asarray(pos, dtype=np.float32)
    z = np.asarray(z)
    emb = np.asarray(emb, dtype=np.float32)
    W_s2n = np.asarray(W_s2n, dtype=np.float32)
    W1 = np.asarray(W1, dtype=np.float32); W2 = np.asarray(W2, dtype=np.float32)
    W3 = np.asarray(W3, dtype=np.float32); W4 = np.asarray(W4, dtype=np.float32)
    Ws = np.asarray(Ws, dtype=np.float32); Wv = np.asarray(Wv, dtype=np.float32)
    Wr1 = np.asarray(Wr1, dtype=np.float32); br1 = np.asarray(br1, dtype=np.float32)
    Wr2 = np.asarray(Wr2, dtype=np.float32); br2 = np.asarray(br2, dtype=np.float32)

    # host prep: embedding lookup folded with input linear
    EW = (emb @ W_s2n) * np.float32(1.0 / np.sqrt(S_MUL))     # [100, 32]
    s0 = EW[z].astype(ml_dtypes.bfloat16)                     # [N, 32] bf16

    # fused transform weights with norm constants folded in
    cs = C_SCALAR * np.float32(1.0 / np.sqrt(S_MUL))
    csb = C_SCALAR * np.float32(INV_SQRT3 / np.sqrt(S_MUL))
    cv = C_VECTOR * np.float32(INV_SQRT3 / np.sqrt(V_MUL))
    wta = np.zeros((3, 80, 80), np.float32)
    wtb = np.zeros((3, 112, 80), np.float32)
    wtc = np.zeros((3, 80, 80), np.float32)
    for l in range(3):
        Wa = cs * (W1[l] @ Ws[l])        # [32,32] s_m -> s
        Wb = csb * (W4[l] @ Ws[l])       # [16,32] v_c*sh_c -> s
        Wc = cv * (W2[l] @ Wv[l])        # [32,16] s*sh_c -> v_c
        Wd = cv * (W3[l] @ Wv[l])        # [16,16] v_c_m -> v_c
        # SSa rows: [s_m(0:32) vx_m(32:48) vy_m(48:64) vz_m(64:80)]
        wta[l, 0:32, 0:32] = Wa
        wta[l, 32:48, 32:48] = Wd
        wta[l, 48:64, 48:64] = Wd
        wta[l, 64:80, 64:80] = Wd
        # SSb rows: [s_y(0:32) waste(32:48) vyy(48:64) s_x(64:96) vxx(96:112)]
        wtb[l, 0:32, 48:64] = Wc
        wtb[l, 48:64, 0:32] = Wb
        wtb[l, 64:96, 32:48] = Wc
        wtb[l, 96:112, 0:32] = Wb
        # SSc rows: [s_z(0:32) waste(32:64) vzz(64:80)]
        wtc[l, 0:32, 64:80] = Wc
        wtc[l, 64:80, 0:32] = Wb

    # L0-only transform: input rows [s_x(0:32) s_y(32:64) s_z(64:96)]
    wl0 = np.zeros((96, 80), np.float32)
    wl0[0:32] = wtb[0, 64:96]      # s_x -> v_x
    wl0[32:64] = wtb[0, 0:32]      # s_y -> v_y
    wl0[64:96] = wtc[0, 0:32]      # s_z -> v_z

    # readout Wr1 with rows permuted to the [s | vx | vy | vz] feature order
    wr1p = np.zeros((80, HID), np.float32)
    wr1p[0:32] = Wr1[0:32]                        # s
    for u in range(V_MUL):
        wr1p[32 + u] = Wr1[S_MUL + 3 * u + 0]     # vx
        wr1p[48 + u] = Wr1[S_MUL + 3 * u + 1]     # vy
        wr1p[64 + u] = Wr1[S_MUL + 3 * u + 2]     # vz

    bdm = np.zeros((128, 128), np.float32)
    for g in range(GPB):
        bdm[g * NA:(g + 1) * NA, g * NA:(g + 1) * NA] = 1.0
    np.fill_diagonal(bdm, 0.0)                    # no self-loops (d2 > 0)
    poolm = np.zeros((128, GPB), np.float32)
    for g in range(GPB):
        poolm[g * NA:(g + 1) * NA, g] = 1.0

    if "nc" not in _CACHE:
        _CACHE["nc"] = _build_program()
    nc = _CACHE["nc"]

    in_maps = []
    for c in range(NCORES):
        psl = pos[c * NPC:(c + 1) * NPC]                       # [8192, 3]
        pb = psl.reshape(NBLK, 128, 3)                         # [64, 128, 3]
        pbt = np.ascontiguousarray(pb.transpose(0, 2, 1))      # [64, 3, 128]
        nrm2 = (pb.astype(np.float64) ** 2).sum(-1).astype(np.float32)
        ph, pm, pl = _split3(pbt)                              # [64, 3, 128] each
        nh, nm_, nl = _split3(nrm2)                            # [64, 128] each
        phf = ph.astype(np.float32); pmf = pm.astype(np.float32)
        plf = pl.astype(np.float32)
        gl = np.zeros((NBLK, 24, 128), ml_dtypes.bfloat16)
        gr = np.zeros((NBLK, 24, 512), ml_dtypes.bfloat16)
        gl[:, 0:3, :] = 1.0
        gr[:, 0, 384:512] = nh
        gr[:, 1, 384:512] = nm_
        gr[:, 2, 384:512] = nl
        for cc in range(3):
            js = slice(cc * 128, (cc + 1) * 128)
            gr[:, 0, js] = ph[:, cc]
            gr[:, 1, js] = pm[:, cc]
            gr[:, 2, js] = pl[:, cc]
            b0 = 3 + 6 * cc
            # rows: (pi coef, diff rhs, d2 rhs)
            gl[:, b0 + 0, :] = ph[:, cc]
            gl[:, b0 + 1, :] = ph[:, cc]
            gl[:, b0 + 2, :] = ph[:, cc]
            gl[:, b0 + 3, :] = pm[:, cc]
            gl[:, b0 + 4, :] = pm[:, cc]
            gl[:, b0 + 5, :] = pl[:, cc]
            for k in (0, 3, 5):
                gr[:, b0 + k, js] = -1.0
            gr[:, b0 + 0, 384:512] = (-2.0 * phf[:, cc]).astype(ml_dtypes.bfloat16)
            gr[:, b0 + 1, 384:512] = (-2.0 * pmf[:, cc]).astype(ml_dtypes.bfloat16)
            gr[:, b0 + 2, 384:512] = (-2.0 * plf[:, cc]).astype(ml_dtypes.bfloat16)
            gr[:, b0 + 3, 384:512] = (-2.0 * phf[:, cc]).astype(ml_dtypes.bfloat16)
            gr[:, b0 + 4, 384:512] = (-2.0 * pmf[:, cc]).astype(ml_dtypes.bfloat16)
            gr[:, b0 + 5, 384:512] = (-2.0 * phf[:, cc]).astype(ml_dtypes.bfloat16)
        gl[:, 21, :] = nh
        gl[:, 22, :] = nm_
        gl[:, 23, :] = nl
        gr[:, 21:24, 384:512] = 1.0
        in_maps.append(dict(
            s0=np.ascontiguousarray(s0[c * NPC:(c + 1) * NPC]),
            gl=gl, gr=gr,
            bd=bdm, wta=wta, wtb=wtb, wtc=wtc, wl0=wl0, poolm=poolm,
            wr1=wr1p, br1=br1.reshape(HID, 1),
            wr2=Wr2, br2=br2.reshape(LAT, 1),
        ))

    res = run_bass_kernel_spmd(nc, in_maps, core_ids=list(range(NCORES)))
    out = np.empty((B, LAT), np.float32)
    for c in range(NCORES):
        out[c * GPC:(c + 1) * GPC] = res.results[c]["outfm"].T
    return out
